# revision 1
# baseline (speedup 1.0000x reference)
"""Trainium2 Bass kernel for nn_CausalSelfAttention_45200235823551.

Causal self-attention with low-rank key/value encoders:
  D=1024, H=16 heads, HD=64, F=32 freqs, KR=3, VR=192, B=2, S=2048.

Sharding: 8 cores, each core owns 2 heads x both batches (tensor parallel
over heads). Each core computes its heads' q/k/v, attention, and a partial
output projection (its heads' rows of Wproj); the host sums the 8 partial
outputs (row-parallel linear unshard).

Per-core layout ("T-major" = feature rows on partitions, sequence on free):
  - xT [1024, 2048] per batch, bf16
  - wcomb [1024, 640] = [Wq_my(128) | Wq_my_pairswap(128) | Wk(192) | Wv(192)]
  - one matmul pass produces qT, qswapT, zkT (key latents), xvT (value latents)
  - k decode: kT = A.T @ zkT with A[192,128] holding complex basis coefs,
    plus a pair-swapped variant for RoPE
  - RoPE applied in T-layout with precomputed cos/sin row tables
  - scoresT[sk,sq] = krotT.T @ qrotT per head (K=64, both heads row-packed)
  - exp on ScalarE (scale=1/8 folded in), causal mask by 0/1 multiply on
    diagonal blocks only
  - attn@v with v stationary: yT_aug[d+1, sq] accumulated over sk blocks;
    an all-ones column of v gives the softmax denominator for free
  - normalize via VectorE reciprocal + GpSimd partition_broadcast
  - partial projection: yout[sq, :] = ynT.T @ Wproj_rows
"""

import os
import sys

import numpy as np

sys.path.insert(0, "/opt/trn_rl_repo")

import ml_dtypes

D, H, HD = 1024, 16, 64
F, KR, VR = 32, 3, 192
B, S = 2, 2048
NCORE = 8
CH = 512          # sq chunk width
NCH = S // CH     # 4
BLK = 128         # sk block
NBLK = S // BLK   # 16
VW = 193          # v_sb per-block: [v_h0(64) | 1 | 1 | zeros(63) | v_h1(64)]
ROPE_BASE = 10000.0

_COMPILED = {}


def _build_bass():
    import concourse.bass as bass
    import concourse.tile as tile
    from concourse import mybir
    from contextlib import ExitStack

    BF = mybir.dt.bfloat16
    F32 = mybir.dt.float32
    AF = mybir.ActivationFunctionType

    nc = bass.Bass()
    xt = nc.dram_tensor("xt", [B, D, S], BF, kind="ExternalInput")
    wcomb = nc.dram_tensor("wcomb", [D, 640], BF, kind="ExternalInput")
    acoef = nc.dram_tensor("acoef", [192, 256], BF, kind="ExternalInput")
    vdeca = nc.dram_tensor("vdeca", [128, VW], BF, kind="ExternalInput")
    vdecb = nc.dram_tensor("vdecb", [128, VW], BF, kind="ExternalInput")
    wproj = nc.dram_tensor("wproj", [128, D], BF, kind="ExternalInput")
    cosT = nc.dram_tensor("cosT", [128, S], BF, kind="ExternalInput")
    sinT = nc.dram_tensor("sinT", [128, S], BF, kind="ExternalInput")
    maskc = nc.dram_tensor("maskc", [128, 4 * CH], BF, kind="ExternalInput")
    yout = nc.dram_tensor("yout", [B, S, D], F32, kind="ExternalOutput")
    dscr = nc.dram_tensor("dscr", [16, CH], F32)  # denom-recip bounce buffer

    with ExitStack() as ctx:
        tc = ctx.enter_context(tile.TileContext(nc))
        consts = ctx.enter_context(tc.tile_pool(name="consts", bufs=1))
        bigs = ctx.enter_context(tc.tile_pool(name="bigs", bufs=2))
        tmps = ctx.enter_context(tc.tile_pool(name="tmps", bufs=3))
        chunks = ctx.enter_context(tc.tile_pool(name="chunks", bufs=3))
        xpool = ctx.enter_context(tc.tile_pool(name="xpool", bufs=16))
        epool = ctx.enter_context(tc.tile_pool(name="epool", bufs=16))
        smalls = ctx.enter_context(tc.tile_pool(name="smalls", bufs=4))
        opool = ctx.enter_context(tc.tile_pool(name="opool", bufs=4))
        sppool = ctx.enter_context(tc.tile_pool(name="sppool", bufs=2, space="PSUM"))
        ypool = ctx.enter_context(tc.tile_pool(name="ypool", bufs=2, space="PSUM"))
        mmpool = ctx.enter_context(tc.tile_pool(name="mmpool", bufs=2, space="PSUM"))

        # ---- load constants ----
        wcomb_sb = []
        for kt in range(8):
            t = consts.tile([128, 640], BF, tag=f"wcomb{kt}")
            nc.sync.dma_start(out=t, in_=wcomb[kt * 128:(kt + 1) * 128, :])
            wcomb_sb.append(t)
        acoef0 = consts.tile([128, 256], BF, tag="acoef0")
        nc.sync.dma_start(out=acoef0, in_=acoef[0:128, :])
        acoef1 = consts.tile([64, 256], BF, tag="acoef1")
        nc.sync.dma_start(out=acoef1, in_=acoef[128:192, :])
        vdeca_sb = consts.tile([128, VW], BF, tag="vdeca")
        nc.sync.dma_start(out=vdeca_sb, in_=vdeca[:, :])
        vdecb_sb = consts.tile([128, VW], BF, tag="vdecb")
        nc.sync.dma_start(out=vdecb_sb, in_=vdecb[:, :])
        wproj_sb = consts.tile([128, D], BF, tag="wproj")
        nc.sync.dma_start(out=wproj_sb, in_=wproj[:, :])
        cos_sb = consts.tile([128, S], BF, tag="cos")
        nc.sync.dma_start(out=cos_sb, in_=cosT[:, :])
        sin_sb = consts.tile([128, S], BF, tag="sin")
        nc.sync.dma_start(out=sin_sb, in_=sinT[:, :])
        mask_sb = consts.tile([128, 4 * CH], BF, tag="mask")
        nc.sync.dma_start(out=mask_sb, in_=maskc[:, :])

        for b in range(B):
            # ---- phase A: projections, k decode, v decode ----
            q_sb = bigs.tile([128, S], BF, tag="q")
            qs_sb = bigs.tile([128, S], BF, tag="qs")
            k_sb = bigs.tile([128, S], BF, tag="k")
            ks_sb = bigs.tile([128, S], BF, tag="ks")
            v_sb = bigs.tile([128, NBLK * VW], BF, tag="v")
            yn_sb = bigs.tile([128, S], BF, tag="yn")

            for c in range(NCH):
                cs = slice(c * CH, (c + 1) * CH)
                xts = []
                for kt in range(8):
                    t = xpool.tile([128, CH], BF, tag="xt")
                    nc.gpsimd.dma_start(
                        out=t, in_=xt[b, kt * 128:(kt + 1) * 128, cs])
                    xts.append(t)
                # combined projection: 5 column tiles of wcomb
                zk0 = chunks.tile([128, CH], BF, tag="zk0")
                zk1 = chunks.tile([64, CH], BF, tag="zk1")
                xva = chunks.tile([128, CH], BF, tag="xva")  # rows 64:128 used
                xvb = chunks.tile([128, CH], BF, tag="xvb")
                for ct in range(5):
                    ps = mmpool.tile([128, CH], F32, tag="mm")
                    for kt in range(8):
                        nc.tensor.matmul(
                            ps, lhsT=wcomb_sb[kt][:, ct * 128:(ct + 1) * 128],
                            rhs=xts[kt], start=(kt == 0), stop=(kt == 7))
                    if ct == 0:
                        nc.vector.tensor_copy(q_sb[:, cs], ps)
                    elif ct == 1:
                        nc.vector.tensor_copy(qs_sb[:, cs], ps)
                    elif ct == 2:
                        nc.vector.tensor_copy(zk0, ps)
                    elif ct == 3:
                        nc.vector.tensor_copy(zk1, ps[0:64, :])
                        nc.vector.tensor_copy(xva[64:128, :], ps[64:128, :])
                    else:
                        nc.vector.tensor_copy(xvb, ps)
                # k decode (and pair-swapped variant)
                psk = mmpool.tile([128, CH], F32, tag="mm")
                nc.tensor.matmul(psk, lhsT=acoef0[:, 0:128], rhs=zk0,
                                 start=True, stop=False)
                nc.tensor.matmul(psk, lhsT=acoef1[:, 0:128], rhs=zk1,
                                 start=False, stop=True)
                nc.vector.tensor_copy(k_sb[:, cs], psk)
                psks = mmpool.tile([128, CH], F32, tag="mm")
                nc.tensor.matmul(psks, lhsT=acoef0[:, 128:256], rhs=zk0,
                                 start=True, stop=False)
                nc.tensor.matmul(psks, lhsT=acoef1[:, 128:256], rhs=zk1,
                                 start=False, stop=True)
                nc.vector.tensor_copy(ks_sb[:, cs], psks)
                # v decode: per 128-seq block, both heads side by side
                for j in range(4):
                    sb = 4 * c + j
                    js = slice(j * BLK, (j + 1) * BLK)
                    psv = mmpool.tile([128, VW], F32, tag="mm")
                    nc.tensor.matmul(psv, lhsT=xva[64:128, js],
                                     rhs=vdeca_sb[64:128, :],
                                     start=True, stop=False)
                    nc.tensor.matmul(psv, lhsT=xvb[:, js], rhs=vdecb_sb,
                                     start=False, stop=True)
                    vs = slice(sb * VW, (sb + 1) * VW)
                    nc.vector.tensor_copy(v_sb[:, vs], psv)
                    nc.vector.memset(v_sb[:, sb * VW + 64:sb * VW + 66], 1.0)

            # ---- RoPE ----
            t1 = tmps.tile([128, S], BF, tag="tmp")
            nc.vector.tensor_mul(t1, q_sb, cos_sb)
            nc.vector.tensor_mul(qs_sb, qs_sb, sin_sb)
            nc.vector.tensor_add(q_sb, t1, qs_sb)
            t2 = tmps.tile([128, S], BF, tag="tmp")
            nc.vector.tensor_mul(t2, k_sb, cos_sb)
            nc.vector.tensor_mul(ks_sb, ks_sb, sin_sb)
            nc.vector.tensor_add(k_sb, t2, ks_sb)

            # ---- phase B: attention + partial projection ----
            for c in range(NCH):
                cs = slice(c * CH, (c + 1) * CH)
                nblk = 4 * (c + 1)
                egroups = {0: [], 1: []}
                for g0 in range(0, nblk, 2):
                    gw = min(2, nblk - g0)
                    for h in (0, 1):
                        hp = slice(h * 64, (h + 1) * 64)
                        sp = sppool.tile([128, 2 * CH], F32, tag="score")
                        for i in range(gw):
                            blk = g0 + i
                            nc.tensor.matmul(
                                sp[:, i * CH:(i + 1) * CH],
                                lhsT=k_sb[hp, blk * BLK:(blk + 1) * BLK],
                                rhs=q_sb[hp, cs], start=True, stop=True)
                        et = epool.tile([128, 2 * CH], BF, tag="exp")
                        nc.scalar.activation(et[:, 0:gw * CH], sp[:, 0:gw * CH],
                                             AF.Exp, scale=0.125)
                        for i in range(gw):
                            blk = g0 + i
                            if blk >= 4 * c:  # diagonal block: causal mask
                                m = blk - 4 * c
                                nc.vector.tensor_mul(
                                    et[:, i * CH:(i + 1) * CH],
                                    et[:, i * CH:(i + 1) * CH],
                                    mask_sb[:, m * CH:(m + 1) * CH])
                        egroups[h].append(et)
                for h in (0, 1):
                    yp = ypool.tile([128, CH], F32, tag="y")
                    if h == 0:
                        oslc, dslc, rslc = slice(0, 65), slice(0, 64), slice(64, 65)
                        vcol = 0, 65
                    else:
                        oslc, dslc, rslc = slice(0, 128), slice(64, 128), slice(0, 1)
                        vcol = 65, 193
                    for blk in range(nblk):
                        et = egroups[h][blk // 2]
                        off = (blk % 2) * CH
                        nc.tensor.matmul(
                            yp[oslc, :],
                            lhsT=v_sb[:, blk * VW + vcol[0]:blk * VW + vcol[1]],
                            rhs=et[:, off:off + CH],
                            start=(blk == 0), stop=(blk == nblk - 1))
                    rc = smalls.tile([128, CH], F32, tag="recip")
                    nc.vector.reciprocal(rc[rslc, :], yp[rslc, :])
                    di = b * 8 + c * 2 + h
                    nc.sync.dma_start(out=dscr[di:di + 1, :], in_=rc[rslc, :])
                    bc = smalls.tile([128, CH], F32, tag="bc")
                    nc.gpsimd.dma_start(
                        out=bc[dslc, :],
                        in_=dscr[di:di + 1, :].to_broadcast([64, CH]))
                    nc.vector.tensor_mul(yn_sb[dslc, cs], yp[dslc, :], bc[dslc, :])
                # partial output projection for this chunk
                for j in range(4):
                    sb = 4 * c + j
                    for n in range(2):
                        pp = mmpool.tile([128, CH], F32, tag="mm")
                        nc.tensor.matmul(
                            pp, lhsT=yn_sb[:, sb * BLK:(sb + 1) * BLK],
                            rhs=wproj_sb[:, n * CH:(n + 1) * CH],
                            start=True, stop=True)
                        ot = opool.tile([128, CH], F32, tag="out")
                        nc.vector.tensor_copy(ot, pp)
                        nc.sync.dma_start(
                            out=yout[b, sb * BLK:(sb + 1) * BLK,
                                     n * CH:(n + 1) * CH],
                            in_=ot)
    _split_dma_waits(nc, mybir)
    return nc


def _split_dma_waits(nc, mybir):
    """This container's walrus rejects instructions whose 64B encoding lacks
    room for their sem waits ("Too many sync wait commands"): DMAs and NoOps
    hold 1 wait, matmuls 2. Hoist excess waits onto a chain of single-wait
    NoOps in the same engine stream directly before the instruction — the
    sequencer blocks on each, which is semantically identical."""
    cap = {}
    f = nc.m.functions[0]
    blocks = f.body if hasattr(f, "body") else f.blocks
    n = 0
    for blk in blocks:
        insts = list(blk.instructions)
        out = []
        changed = False
        for inst in insts:
            si = inst.sync_info
            tn = type(inst).__name__
            limit = cap.get(tn, 1)
            if si is not None and si.on_wait and len(si.on_wait) > limit:
                waits = list(si.on_wait)
                keep = waits[-limit:]
                for w in waits[:-limit]:
                    nop = mybir.InstNoOp(name=f"I-dmaw-{n}")
                    n += 1
                    nop.engine = inst.engine
                    nop.sync_info = mybir.SyncInfo(on_wait=[w], on_update=[])
                    nc.register_instruction(nop)
                    out.append(nop)
                inst.sync_info = mybir.SyncInfo(
                    on_wait=keep, on_update=list(si.on_update or []))
                changed = True
            out.append(inst)
        if changed:
            if hasattr(blk, "set_instructions"):
                blk.set_instructions(out)
            else:
                try:
                    blk.instructions = out
                except Exception:
                    blk.instructions[:] = out
    return nc


def _host_inputs(x, Wq, Wk, Wv, key_decoder, value_decoder, Wproj):
    bf16 = ml_dtypes.bfloat16
    x = np.asarray(x, np.float32)
    Wq = np.asarray(Wq, np.float32)
    Wk = np.asarray(Wk, np.float32)
    Wv = np.asarray(Wv, np.float32)
    key_decoder = np.asarray(key_decoder, np.float32)
    value_decoder = np.asarray(value_decoder, np.float32)
    Wproj = np.asarray(Wproj, np.float32)

    xt = np.ascontiguousarray(x.transpose(0, 2, 1)).astype(bf16)  # [B, D, S]

    half = HD // 2
    freq = 1.0 / (ROPE_BASE ** (np.arange(half, dtype=np.float32) / half))
    th = np.outer(np.arange(S, dtype=np.float32), freq)  # [S, 32]
    cos, sin = np.cos(th), np.sin(th)
    rows = np.arange(128)
    fidx = (rows % 64) // 2
    cosT = cos[:, fidx].T.astype(bf16)                       # [128, S]
    sgn = np.where(rows % 2 == 0, -1.0, 1.0)[:, None]
    sinT = (sin[:, fidx].T * sgn).astype(bf16)

    maskc = np.zeros((128, 4 * CH), np.float32)
    p = np.arange(128)[:, None]
    j = np.arange(CH)[None, :]
    for m in range(4):
        maskc[:, m * CH:(m + 1) * CH] = (p <= j - 128 * m)
    maskc = maskc.astype(bf16)

    Wq4 = Wq.reshape(D, H, HD)
    br, bi = key_decoder[..., 0], key_decoder[..., 1]  # [F, H, KR]

    in_maps = []
    for core in range(NCORE):
        h0, h1 = 2 * core, 2 * core + 1
        wq_my = Wq4[:, [h0, h1], :].reshape(D, 128)
        wq_sw = np.ascontiguousarray(
            Wq4[:, [h0, h1], :].reshape(D, 2, 32, 2)[..., ::-1]).reshape(D, 128)
        wcomb = np.concatenate([wq_my, wq_sw, Wk, Wv], axis=1).astype(bf16)

        A = np.zeros((192, 128), np.float32)
        for hl, h in enumerate((h0, h1)):
            for f in range(F):
                for r in range(KR):
                    A[f * 6 + r * 2 + 0, hl * 64 + 2 * f] = br[f, h, r]
                    A[f * 6 + r * 2 + 1, hl * 64 + 2 * f] = -bi[f, h, r]
                    A[f * 6 + r * 2 + 0, hl * 64 + 2 * f + 1] = bi[f, h, r]
                    A[f * 6 + r * 2 + 1, hl * 64 + 2 * f + 1] = br[f, h, r]
        Asw = np.ascontiguousarray(
            A.reshape(192, 2, 32, 2)[..., ::-1]).reshape(192, 128)
        acoef = np.concatenate([A, Asw], axis=1).astype(bf16)

        vdeca = np.zeros((128, VW), np.float32)
        vdecb = np.zeros((128, VW), np.float32)
        vdeca[64:128, 0:64] = value_decoder[h0][0:64, :]
        vdeca[64:128, 129:193] = value_decoder[h1][0:64, :]
        vdecb[:, 0:64] = value_decoder[h0][64:192, :]
        vdecb[:, 129:193] = value_decoder[h1][64:192, :]

        wproj_my = np.concatenate(
            [Wproj[h0 * 64:(h0 + 1) * 64, :], Wproj[h1 * 64:(h1 + 1) * 64, :]],
            axis=0).astype(bf16)

        in_maps.append({
            "xt": xt, "wcomb": wcomb, "acoef": acoef,
            "vdeca": vdeca.astype(bf16), "vdecb": vdecb.astype(bf16),
            "wproj": wproj_my, "cosT": cosT, "sinT": sinT, "maskc": maskc,
        })
    return in_maps


def kernel(x, Wq, Wk, Wv, key_decoder, value_decoder, Wproj):
    from concourse.bass_utils import run_bass_kernel_spmd

    if "nc" not in _COMPILED:
        _COMPILED["nc"] = _build_bass()
    nc = _COMPILED["nc"]

    in_maps = _host_inputs(x, Wq, Wk, Wv, key_decoder, value_decoder, Wproj)
    import time as _time
    t0 = _time.time()
    res = run_bass_kernel_spmd(nc, in_maps, list(range(NCORE)))
    _COMPILED["exec_wall_ns"] = (_time.time() - t0) * 1e9
    _COMPILED["last_result"] = res
    out = np.zeros((B, S, D), np.float64)
    for r in res.results:
        out += r["yout"].astype(np.float64)
    return out.astype(np.float32)



# revision 20
# speedup vs baseline: 1.7227x; 1.7227x over previous
"""Trainium2 Bass kernel for nn_CausalSelfAttention_45200235823551.

Causal self-attention with low-rank key/value encoders:
  D=1024, H=16 heads, HD=64, F=32 freqs, KR=3, VR=192, B=2, S=2048.

Sharding: 8 cores = 2 batches x 4 head-groups. Core i owns batch i//4 and
heads 4*(i%4)..4*(i%4)+3. Each core computes its heads' q/k/v, attention,
and a partial output projection (its heads' rows of Wproj); the host sums
the 4 partials per batch (row-parallel linear unshard).

Per-core layout ("T-major": feature rows on partitions, sequence on free):
  - xT [1024, 2048] bf16, loaded one 512-wide chunk per DMA
  - wcomb [1024, 704] = [Wq(2x128) | Wk@A key-fold (2x128) | Wv (128+64)]
    The key decoder is folded into the projection on the host:
    kT = A^T (Wk^T x) = (Wk A)^T x, so no separate decode stage.
  - RoPE pair-swap via a 128x128 permutation matmul (PE), then
    qrot = q*cos + (Pq)*sin on DVE with sign folded into the sin table
  - scoresT[sk,sq] = krotT.T @ qrotT per head (K=64)
  - causal N-shrink: diagonal block m only computes columns >= 128m;
    triangular mask is a [128,128] multiply on the first 128 columns
  - attn@v with v stationary: per head the 128-wide lhsT is
    [v(64) | ones(64)] so rows 64:128 of the accumulator replicate the
    softmax denominator; normalize = one partition-shifted DVE divide
  - partial projection: yout[sq,:] = yn.T @ Wproj_rows, bf16 partials
"""

import os
import sys

import numpy as np

sys.path.insert(0, "/opt/trn_rl_repo")

import ml_dtypes

D, H, HD = 1024, 16, 64
F, KR, VR = 32, 3, 192
B, S = 2, 2048
NCORE = 8
CH = 512          # sq chunk width
NCH = S // CH     # 4
BLK = 128         # sk block
VW = 384          # v_sb per-block: [v_h0|ones|v_h1 | v_h2|ones|v_h3]
ROPE_BASE = 10000.0

_COMPILED = {}


def _build_bass():
    import concourse.bass as bass
    import concourse.tile as tile
    from concourse import mybir
    from contextlib import ExitStack

    BF = mybir.dt.bfloat16
    F32 = mybir.dt.float32
    AF = mybir.ActivationFunctionType
    DIV = mybir.AluOpType.divide

    nc = bass.Bass()
    xt = nc.dram_tensor("xt", [D, S], BF, kind="ExternalInput")
    wcomb = nc.dram_tensor("wcomb", [D, 704], BF, kind="ExternalInput")
    permw = nc.dram_tensor("permw", [128, 128], BF, kind="ExternalInput")
    vdeca = nc.dram_tensor("vdeca", [128, VW], BF, kind="ExternalInput")
    vdecb = nc.dram_tensor("vdecb", [128, VW], BF, kind="ExternalInput")
    wproj = nc.dram_tensor("wproj", [256, D], BF, kind="ExternalInput")
    cosT = nc.dram_tensor("cosT", [128, S], BF, kind="ExternalInput")
    sinT = nc.dram_tensor("sinT", [128, S], BF, kind="ExternalInput")
    trim = nc.dram_tensor("trim", [128, 128], BF, kind="ExternalInput")
    yout = nc.dram_tensor("yout", [S, D], BF, kind="ExternalOutput")

    with ExitStack() as ctx:
        tc = ctx.enter_context(tile.TileContext(nc))
        consts = ctx.enter_context(tc.tile_pool(name="consts", bufs=1))
        bigs = ctx.enter_context(tc.tile_pool(name="bigs", bufs=1))
        xpool = ctx.enter_context(tc.tile_pool(name="xpool", bufs=2))
        xvpool = ctx.enter_context(tc.tile_pool(name="xvpool", bufs=2))
        tmps = ctx.enter_context(tc.tile_pool(name="tmps", bufs=4))
        ynpool = ctx.enter_context(tc.tile_pool(name="ynpool", bufs=4))
        epool = ctx.enter_context(tc.tile_pool(name="epool", bufs=18))
        otpool = ctx.enter_context(tc.tile_pool(name="otpool", bufs=3))
        mmpool = ctx.enter_context(tc.tile_pool(name="mmpool", bufs=2, space="PSUM"))
        sppool = ctx.enter_context(tc.tile_pool(name="sppool", bufs=2, space="PSUM"))
        ypool = ctx.enter_context(tc.tile_pool(name="ypool", bufs=2, space="PSUM"))

        # ---- input prefetch + constants, ordered for fastest PE start ----
        def load_x(c):
            cs = slice(c * CH, (c + 1) * CH)
            t = xpool.tile([128, 8 * CH], BF, tag="xts", name="xts")
            nc.sync.dma_start(
                out=t[:, :].rearrange("p (k j) -> p k j", k=8, j=CH),
                in_=xt[:, cs].rearrange("(k p) j -> p k j", k=8, p=128))
            return t

        # chunk 0: interleave per-ktile x slices with wcomb tiles so the
        # first proj matmuls start as early as possible
        xts0 = xpool.tile([128, 8 * CH], BF, tag="xts", name="xts0")
        wcomb_sb = []
        for kt in range(8):
            t = consts.tile([128, 704], BF, tag=f"wcomb{kt}")
            nc.sync.dma_start(out=t, in_=wcomb[kt * 128:(kt + 1) * 128, :])
            wcomb_sb.append(t)
            nc.sync.dma_start(
                out=xts0[:, kt * CH:(kt + 1) * CH],
                in_=xt[kt * 128:(kt + 1) * 128, 0:CH])
        xts_pending = [xts0]
        perm_sb = consts.tile([128, 128], BF, tag="perm")
        nc.sync.dma_start(out=perm_sb, in_=permw[:, :])
        cos_sb = consts.tile([128, S], BF, tag="cos")
        nc.sync.dma_start(out=cos_sb, in_=cosT[:, :])
        sin_sb = consts.tile([128, S], BF, tag="sin")
        nc.sync.dma_start(out=sin_sb, in_=sinT[:, :])
        vdeca_sb = consts.tile([128, VW], BF, tag="vdeca")
        nc.sync.dma_start(out=vdeca_sb, in_=vdeca[:, :])
        vdecb_sb = consts.tile([128, VW], BF, tag="vdecb")
        nc.sync.dma_start(out=vdecb_sb, in_=vdecb[:, :])
        tri_sb = consts.tile([128, 128], BF, tag="tri")
        nc.sync.dma_start(out=tri_sb, in_=trim[:, :])
        wp_sb = []
        for i in range(2):
            t = consts.tile([128, D], BF, tag=f"wp{i}")
            nc.sync.dma_start(out=t, in_=wproj[i * 128:(i + 1) * 128, :])
            wp_sb.append(t)

        # persistent per-core tensors
        qrot = [bigs.tile([128, S], BF, tag=f"qrot{i}", name=f"qrot{i}")
                for i in range(2)]
        krot = [bigs.tile([128, S], BF, tag=f"krot{i}", name=f"krot{i}")
                for i in range(2)]
        v_sb = bigs.tile([128, 16 * VW], BF, tag="v")

        def phase_a_units(c):
            """Generator: projection / rope / v-decode for chunk c, one
            PE-sized unit of work per yield."""
            cs = slice(c * CH, (c + 1) * CH)
            xts = xts_pending.pop(0)
            if c + 1 < NCH:
                xts_pending.append(load_x(c + 1))
            xvab = xvpool.tile([128, CH], BF, tag="xvab", name="xvab")
            xvb2 = xvpool.tile([128, CH], BF, tag="xvb2", name="xvb2")
            for ct in range(6):
                ps = mmpool.tile([128, CH], F32, tag="mm", name="mm")
                m = 64 if ct == 5 else 128
                for kt in range(8):
                    nc.tensor.matmul(
                        ps[0:m, :],
                        lhsT=wcomb_sb[kt][:, ct * 128:ct * 128 + m],
                        rhs=xts[:, kt * CH:(kt + 1) * CH],
                        start=(kt == 0), stop=(kt == 7))
                if ct == 0:
                    nc.vector.tensor_copy(qrot[0][:, cs], ps)
                elif ct == 1:
                    nc.vector.tensor_copy(qrot[1][:, cs], ps)
                elif ct == 2:
                    nc.vector.tensor_copy(krot[0][:, cs], ps)
                elif ct == 3:
                    nc.vector.tensor_copy(krot[1][:, cs], ps)
                elif ct == 4:
                    nc.vector.tensor_copy(xvab, ps)
                else:
                    nc.vector.tensor_copy(xvb2[0:64, :], ps[0:64, :])
                    nc.vector.memset(xvb2[64:65, :], 1.0)
                yield
            # v decode: per 128-seq block, 4 heads side by side
            for j in range(4):
                sb = 4 * c + j
                js = slice(j * BLK, (j + 1) * BLK)
                psv = ypool.tile([128, CH], F32, tag="yp", name="psv")
                nc.tensor.matmul(psv[:, 0:VW], lhsT=xvab[:, js], rhs=vdeca_sb,
                                 start=True, stop=False)
                nc.tensor.matmul(psv[:, 0:VW], lhsT=xvb2[0:65, js],
                                 rhs=vdecb_sb[0:65, :], start=False, stop=True)
                nc.scalar.copy(v_sb[:, sb * VW:(sb + 1) * VW], psv[:, 0:VW])
                yield
            # RoPE: T <- T*cos + (P T)*sin, in T-layout per 512 chunk
            for tt in (qrot[0], qrot[1], krot[0], krot[1]):
                pq = mmpool.tile([128, CH], F32, tag="mm", name="mm")
                nc.tensor.matmul(pq, lhsT=perm_sb, rhs=tt[:, cs],
                                 start=True, stop=True)
                t1 = tmps.tile([128, CH], BF, tag="t1", name="t1")
                nc.vector.tensor_mul(t1, pq, sin_sb[:, cs])
                t2 = tmps.tile([128, CH], BF, tag="t2", name="t2")
                nc.vector.tensor_mul(t2, tt[:, cs], cos_sb[:, cs])
                nc.vector.tensor_add(tt[:, cs], t1, t2)
                yield

        def scores_units(c, h, ets):
            """Generator: scores + exp (+ causal mask) for head h, chunk c.
            Appends et tiles to ets; one psum tile per yield."""
            hp, hl = h // 2, h % 2
            rows = slice(hl * 64, (hl + 1) * 64)
            kk, qq = krot[hp], qrot[hp]
            cs = slice(c * CH, (c + 1) * CH)
            for g in range(2 * c):   # full block pairs
                sp = sppool.tile([128, 2 * CH], F32, tag="sp", name="sp")
                for i in range(2):
                    blk = 2 * g + i
                    nc.tensor.matmul(
                        sp[:, i * CH:(i + 1) * CH],
                        lhsT=kk[rows, blk * BLK:(blk + 1) * BLK],
                        rhs=qq[rows, cs], start=True, stop=True)
                et = epool.tile([128, 2 * CH], BF, tag="et", name="et")
                nc.scalar.activation(et, sp, AF.Exp, scale=0.125)
                ets.append(et)
                yield
            # diagonal blocks, N-shrunk: m covers cols [128m:512] of the chunk
            spd0 = sppool.tile([128, 2 * CH], F32, tag="sp", name="sp")
            nc.tensor.matmul(
                spd0[:, 0:512],
                lhsT=kk[rows, (4 * c) * BLK:(4 * c + 1) * BLK],
                rhs=qq[rows, cs], start=True, stop=True)
            nc.tensor.matmul(
                spd0[:, 512:896],
                lhsT=kk[rows, (4 * c + 1) * BLK:(4 * c + 2) * BLK],
                rhs=qq[rows, c * CH + 128:(c + 1) * CH], start=True, stop=True)
            etd0 = epool.tile([128, 2 * CH], BF, tag="et", name="et")
            nc.scalar.activation(etd0[:, 0:896], spd0[:, 0:896],
                                 AF.Exp, scale=0.125)
            nc.gpsimd.tensor_mul(etd0[:, 0:128], etd0[:, 0:128], tri_sb)
            nc.gpsimd.tensor_mul(etd0[:, 512:640], etd0[:, 512:640], tri_sb)
            ets.append(etd0)
            yield
            spd1 = sppool.tile([128, 2 * CH], F32, tag="sp", name="sp")
            nc.tensor.matmul(
                spd1[:, 0:256],
                lhsT=kk[rows, (4 * c + 2) * BLK:(4 * c + 3) * BLK],
                rhs=qq[rows, c * CH + 256:(c + 1) * CH], start=True, stop=True)
            nc.tensor.matmul(
                spd1[:, 256:384],
                lhsT=kk[rows, (4 * c + 3) * BLK:(4 * c + 4) * BLK],
                rhs=qq[rows, c * CH + 384:(c + 1) * CH], start=True, stop=True)
            etd1 = epool.tile([128, 2 * CH], BF, tag="et", name="et")
            nc.scalar.activation(etd1[:, 0:384], spd1[:, 0:384],
                                 AF.Exp, scale=0.125)
            nc.gpsimd.tensor_mul(etd1[:, 0:128], etd1[:, 0:128], tri_sb)
            nc.gpsimd.tensor_mul(etd1[:, 256:384], etd1[:, 256:384], tri_sb)
            ets.append(etd1)
            yield

        def attnv_units(c, h, ets, yn):
            hp, hl = h // 2, h % 2
            base = hp * 192 + hl * 64
            yp = ypool.tile([128, CH], F32, tag="yp", name="yp")
            for blk in range(4 * c):
                et = ets[blk // 2]
                off = (blk % 2) * CH
                nc.tensor.matmul(
                    yp, lhsT=v_sb[:, blk * VW + base:blk * VW + base + 128],
                    rhs=et[:, off:off + CH],
                    start=(blk == 0), stop=False)
                if blk % 2 == 1:
                    yield
            etd0, etd1 = ets[-2], ets[-1]
            # diag m writes cols [128m:512]; one start (first write) and one
            # stop (last write) per psum bank, column completion is tracked
            # by the tile framework's write deps
            dspec = [
                (0, etd0, 0, 512),
                (1, etd0, 512, 384),
                (2, etd1, 0, 256),
                (3, etd1, 256, 128),
            ]
            for m, etd, off, w in dspec:
                blk = 4 * c + m
                vsl = v_sb[:, blk * VW + base:blk * VW + base + 128]
                nc.tensor.matmul(
                    yp[:, m * 128:CH],
                    lhsT=vsl, rhs=etd[:, off:off + w],
                    start=(c == 0 and m == 0), stop=(m == 3))
                if m % 2 == 1:
                    yield
            # normalize: y * 1/denom. The 64 ones-columns replicated the
            # denominator onto the other 64 rows; reciprocal shifts it back
            # onto y's partitions (single-input ops may cross partitions,
            # tensor_tensor inputs may not).
            if hl == 0:
                yrows, drows = slice(0, 64), slice(64, 128)
            else:
                yrows, drows = slice(64, 128), slice(0, 64)
            rc = tmps.tile([128, CH], F32, tag="rc", name="rc")
            nc.vector.reciprocal(rc[yrows, :], yp[drows, :])
            nc.vector.tensor_mul(yn[hp][yrows, :], yp[yrows, :], rc[yrows, :])
            yield

        def b_units(c, yn):
            """Attention for chunk c: pair head h's scores with head h-1's
            attn@v so PE alternates between them while exps drain."""
            prev = None
            for h in range(4):
                ets = []
                sg = scores_units(c, h, ets)
                ag = attnv_units(c, prev[0], prev[1], yn) if prev else None
                for _ in sg:
                    yield
                    if ag is not None and next(ag, "done") != "done":
                        yield
                if ag is not None:
                    for _ in ag:
                        yield
                prev = (h, ets)
            for _ in attnv_units(c, prev[0], prev[1], yn):
                yield

        def out_proj_units(c, yn):
            for j in range(4):
                sb = 4 * c + j
                js = slice(j * BLK, (j + 1) * BLK)
                ot = otpool.tile([128, D], BF, tag="ot", name="ot")
                for n in range(2):
                    pp = mmpool.tile([128, CH], F32, tag="mm", name="mm")
                    nc.tensor.matmul(pp, lhsT=yn[0][:, js],
                                     rhs=wp_sb[0][:, n * CH:(n + 1) * CH],
                                     start=True, stop=False)
                    nc.tensor.matmul(pp, lhsT=yn[1][:, js],
                                     rhs=wp_sb[1][:, n * CH:(n + 1) * CH],
                                     start=False, stop=True)
                    nc.vector.tensor_copy(ot[:, n * CH:(n + 1) * CH], pp)
                    yield
                nc.sync.dma_start(
                    out=yout[sb * BLK:(sb + 1) * BLK, :], in_=ot)

        def drain_interleaved(gb, gas, ratio):
            """Emit all of gb, interspersing units from the ga generators at
            the given rate; then drain the gas."""
            acc = 0.0
            for _ in gb:
                acc += ratio
                while acc >= 1.0 and gas:
                    if next(gas[0], "done") == "done":
                        gas.pop(0)
                    else:
                        acc -= 1.0
            for ga in gas:
                for _ in ga:
                    pass

        # chunk 0 phase A runs alone (nothing to overlap with yet).
        # Output projections for chunks 0..2 are all deferred into the last
        # chunk's attention window: that window is Activation-bound (no next
        # phase A left to interleave), so it needs the PE filler the most.
        for _ in phase_a_units(0):
            pass
        yns = []
        for c in range(NCH):
            yn = [ynpool.tile([128, CH], BF, tag=f"yn{i}", name=f"yn{i}")
                  for i in range(2)]
            yns.append(yn)
            gas = []
            na = 0
            if c + 1 < NCH:
                gas.append(phase_a_units(c + 1))
                na += 14
            else:
                for cc in range(NCH - 1):
                    gas.append(out_proj_units(cc, yns[cc]))
                    na += 8
            nb = 4 * (4 * c + 7) + 1
            drain_interleaved(b_units(c, yn), gas, na / nb)
        for _ in out_proj_units(NCH - 1, yns[-1]):
            pass

    _split_dma_waits(nc, mybir)
    return nc


def _split_dma_waits(nc, mybir):
    """This container's walrus rejects instructions whose 64B encoding lacks
    room for their sem waits ("Too many sync wait commands"): DMAs and NoOps
    hold 1 wait, matmuls 2. Hoist excess waits onto a chain of single-wait
    NoOps in the same engine stream directly before the instruction — the
    sequencer blocks on each, which is semantically identical."""
    cap = {}
    f = nc.m.functions[0]
    blocks = f.body if hasattr(f, "body") else f.blocks
    n = 0
    for blk in blocks:
        insts = list(blk.instructions)
        out = []
        changed = False
        for inst in insts:
            si = inst.sync_info
            tn = type(inst).__name__
            limit = cap.get(tn, 1)
            if si is not None and si.on_wait and len(si.on_wait) > limit:
                waits = list(si.on_wait)
                keep = waits[-limit:]
                for w in waits[:-limit]:
                    nop = mybir.InstNoOp(name=f"I-dmaw-{n}")
                    n += 1
                    nop.engine = inst.engine
                    nop.sync_info = mybir.SyncInfo(on_wait=[w], on_update=[])
                    nc.register_instruction(nop)
                    out.append(nop)
                inst.sync_info = mybir.SyncInfo(
                    on_wait=keep, on_update=list(si.on_update or []))
                changed = True
            out.append(inst)
        if changed:
            if hasattr(blk, "set_instructions"):
                blk.set_instructions(out)
            else:
                try:
                    blk.instructions = out
                except Exception:
                    blk.instructions[:] = out
    return nc


def _host_inputs(x, Wq, Wk, Wv, key_decoder, value_decoder, Wproj):
    bf16 = ml_dtypes.bfloat16
    x = np.asarray(x, np.float32)
    Wq = np.asarray(Wq, np.float32)
    Wk = np.asarray(Wk, np.float32)
    Wv = np.asarray(Wv, np.float32)
    key_decoder = np.asarray(key_decoder, np.float32)
    value_decoder = np.asarray(value_decoder, np.float32)
    Wproj = np.asarray(Wproj, np.float32)

    xts = [np.ascontiguousarray(x[b].T).astype(bf16) for b in range(B)]

    half = HD // 2
    freq = 1.0 / (ROPE_BASE ** (np.arange(half, dtype=np.float32) / half))
    th = np.outer(np.arange(S, dtype=np.float32), freq)  # [S, 32]
    cos, sin = np.cos(th), np.sin(th)
    rows = np.arange(128)
    fidx = (rows % 64) // 2
    cosT = cos[:, fidx].T.astype(bf16)                       # [128, S]
    sgn = np.where(rows % 2 == 0, -1.0, 1.0)[:, None]
    sinT = (sin[:, fidx].T * sgn).astype(bf16)

    p = np.arange(128)
    permw = np.zeros((128, 128), np.float32)
    permw[p, p ^ 1] = 1.0       # pair swap
    permw = permw.astype(bf16)

    trim = (p[:, None] <= p[None, :]).astype(np.float32).astype(bf16)

    Wq4 = Wq.reshape(D, H, HD)
    br, bi = key_decoder[..., 0], key_decoder[..., 1]  # [F, H, KR]

    in_maps = []
    for core in range(NCORE):
        b, hg = core // 4, core % 4
        hh = [4 * hg + i for i in range(4)]

        wq0 = Wq4[:, [hh[0], hh[1]], :].reshape(D, 128)
        wq1 = Wq4[:, [hh[2], hh[3]], :].reshape(D, 128)
        # key decoder folded into Wk: kT = A^T Wk^T x = (Wk A)^T x
        kf = []
        for hp in range(2):
            A = np.zeros((192, 128), np.float32)
            for hl, h in enumerate((hh[2 * hp], hh[2 * hp + 1])):
                for f in range(F):
                    for r in range(KR):
                        A[f * 6 + r * 2 + 0, hl * 64 + 2 * f] = br[f, h, r]
                        A[f * 6 + r * 2 + 1, hl * 64 + 2 * f] = -bi[f, h, r]
                        A[f * 6 + r * 2 + 0, hl * 64 + 2 * f + 1] = bi[f, h, r]
                        A[f * 6 + r * 2 + 1, hl * 64 + 2 * f + 1] = br[f, h, r]
            kf.append(Wk @ A)    # [D, 128]
        wcomb = np.concatenate(
            [wq0, wq1, kf[0], kf[1], Wv], axis=1).astype(bf16)  # [D, 704]

        vdeca = np.zeros((128, VW), np.float32)
        vdecb = np.zeros((128, VW), np.float32)
        for i, h in enumerate(hh):
            hp, hl = i // 2, i % 2
            c0 = hp * 192 + hl * 128   # 0, 128, 192, 320
            vdeca[:, c0:c0 + 64] = value_decoder[h][0:128, :]
            vdecb[0:64, c0:c0 + 64] = value_decoder[h][128:192, :]
        vdecb[64, 64:128] = 1.0      # ones indicator rows
        vdecb[64, 256:320] = 1.0

        wproj_my = np.concatenate(
            [Wproj[h * 64:(h + 1) * 64, :] for h in hh], axis=0).astype(bf16)

        in_maps.append({
            "xt": xts[b], "wcomb": wcomb, "permw": permw,
            "vdeca": vdeca.astype(bf16), "vdecb": vdecb.astype(bf16),
            "wproj": wproj_my, "cosT": cosT, "sinT": sinT, "trim": trim,
        })
    return in_maps


def kernel(x, Wq, Wk, Wv, key_decoder, value_decoder, Wproj):
    from concourse.bass_utils import run_bass_kernel_spmd

    if "nc" not in _COMPILED:
        _COMPILED["nc"] = _build_bass()
    nc = _COMPILED["nc"]

    in_maps = _host_inputs(x, Wq, Wk, Wv, key_decoder, value_decoder, Wproj)
    import time as _time
    t0 = _time.time()
    res = run_bass_kernel_spmd(nc, in_maps, list(range(NCORE)))
    _COMPILED["exec_wall_ns"] = (_time.time() - t0) * 1e9
    _COMPILED["last_result"] = res
    out = np.zeros((B, S, D), np.float32)
    for core in range(NCORE):
        out[core // 4] += res.results[core]["yout"].astype(np.float32)
    return out


# revision 51
# speedup vs baseline: 1.7735x; 1.0295x over previous
"""Trainium2 Bass kernel for nn_CausalSelfAttention_45200235823551.

Causal self-attention with low-rank key/value encoders:
  D=1024, H=16 heads, HD=64, F=32 freqs, KR=3, VR=192, B=2, S=2048.

Sharding: 8 cores = 2 batches x 4 head-groups. Core i owns batch i//4 and
heads 4*(i%4)..4*(i%4)+3. Each core computes its heads' q/k/v, attention,
and a partial output projection (its heads' rows of Wproj); the host sums
the 4 partials per batch (row-parallel linear unshard).

Per-core layout ("T-major": feature rows on partitions, sequence on free):
  - xT [1024, 2048] bf16, loaded one 512-wide chunk per DMA
  - wcomb [1024, 704] = [Wq(2x128) | Wk@A key-fold (2x128) | Wv (128+64)]
    The key decoder is folded into the projection on the host:
    kT = A^T (Wk^T x) = (Wk A)^T x, so no separate decode stage.
  - RoPE pair-swap via a 128x128 permutation matmul (PE), then
    qrot = q*cos + (Pq)*sin on DVE with sign folded into the sin table
  - scoresT[sk,sq] = krotT.T @ qrotT per head (K=64)
  - causal N-shrink: diagonal block m only computes columns >= 128m;
    triangular mask is a [128,128] multiply on the first 128 columns
  - attn@v with v stationary: per head the 128-wide lhsT is
    [v(64) | ones(64)] so rows 64:128 of the accumulator replicate the
    softmax denominator; normalize = one partition-shifted DVE divide
  - partial projection: yout[sq,:] = yn.T @ Wproj_rows, bf16 partials
"""

import os
import sys

import numpy as np

sys.path.insert(0, "/opt/trn_rl_repo")

import ml_dtypes

D, H, HD = 1024, 16, 64
F, KR, VR = 32, 3, 192
B, S = 2, 2048
NCORE = 8
CH = 512          # sq chunk width
NCH = S // CH     # 4
BLK = 128         # sk block
VW = 384          # v_sb per-block: [v_h0|ones|v_h1 | v_h2|ones|v_h3]
ROPE_BASE = 10000.0

_COMPILED = {}


def _build_bass():
    import concourse.bass as bass
    import concourse.tile as tile
    from concourse import mybir
    from contextlib import ExitStack

    BF = mybir.dt.bfloat16
    F32 = mybir.dt.float32
    AF = mybir.ActivationFunctionType
    DIV = mybir.AluOpType.divide

    nc = bass.Bass()
    xt = nc.dram_tensor("xt", [D, S], BF, kind="ExternalInput")
    wcomb = nc.dram_tensor("wcomb", [D, 704], BF, kind="ExternalInput")
    permw = nc.dram_tensor("permw", [128, 128], BF, kind="ExternalInput")
    vdeca = nc.dram_tensor("vdeca", [128, VW], BF, kind="ExternalInput")
    vdecb = nc.dram_tensor("vdecb", [128, VW], BF, kind="ExternalInput")
    wproj = nc.dram_tensor("wproj", [256, D], BF, kind="ExternalInput")
    cosT = nc.dram_tensor("cosT", [128, S], BF, kind="ExternalInput")
    sinT = nc.dram_tensor("sinT", [128, S], BF, kind="ExternalInput")
    trim = nc.dram_tensor("trim", [128, 128], BF, kind="ExternalInput")
    yout = nc.dram_tensor("yout", [S, D], BF, kind="ExternalOutput")

    with ExitStack() as ctx:
        tc = ctx.enter_context(tile.TileContext(nc))
        consts = ctx.enter_context(tc.tile_pool(name="consts", bufs=1))
        bigs = ctx.enter_context(tc.tile_pool(name="bigs", bufs=1))
        xpool = ctx.enter_context(tc.tile_pool(name="xpool", bufs=2))
        xvpool = ctx.enter_context(tc.tile_pool(name="xvpool", bufs=2))
        tmps = ctx.enter_context(tc.tile_pool(name="tmps", bufs=4))
        ynpool = ctx.enter_context(tc.tile_pool(name="ynpool", bufs=1))
        epool = ctx.enter_context(tc.tile_pool(name="epool", bufs=26))
        otpool = ctx.enter_context(tc.tile_pool(name="otpool", bufs=7))
        mmpool = ctx.enter_context(tc.tile_pool(name="mmpool", bufs=2, space="PSUM"))
        sppool = ctx.enter_context(tc.tile_pool(name="sppool", bufs=2, space="PSUM"))
        ypool = ctx.enter_context(tc.tile_pool(name="ypool", bufs=2, space="PSUM"))

        # ---- input prefetch + constants, ordered for fastest PE start ----
        def load_x(c):
            cs = slice(c * CH, (c + 1) * CH)
            t = xpool.tile([128, 8 * CH], BF, tag="xts", name="xts")
            nc.sync.dma_start(
                out=t[:, :].rearrange("p (k j) -> p k j", k=8, j=CH),
                in_=xt[:, cs].rearrange("(k p) j -> p k j", k=8, p=128))
            return t

        # chunk 0: x first (the longest transfer), weight tiles stream
        # in behind it and the first ct group consumes them as they land
        xts_pending = [load_x(0)]
        wcomb_sb = []
        for kt in range(8):
            t = consts.tile([128, 704], BF, tag=f"wcomb{kt}")
            nc.sync.dma_start(out=t, in_=wcomb[kt * 128:(kt + 1) * 128, :])
            wcomb_sb.append(t)
        perm_sb = consts.tile([128, 128], BF, tag="perm")
        nc.sync.dma_start(out=perm_sb, in_=permw[:, :])
        cos_sb = consts.tile([128, S], BF, tag="cos")
        nc.sync.dma_start(out=cos_sb, in_=cosT[:, :])
        sin_sb = consts.tile([128, S], BF, tag="sin")
        nc.sync.dma_start(out=sin_sb, in_=sinT[:, :])
        vdeca_sb = consts.tile([128, VW], BF, tag="vdeca")
        nc.sync.dma_start(out=vdeca_sb, in_=vdeca[:, :])
        vdecb_sb = consts.tile([128, VW], BF, tag="vdecb")
        nc.sync.dma_start(out=vdecb_sb, in_=vdecb[:, :])
        tri_sb = consts.tile([128, 128], BF, tag="tri")
        nc.sync.dma_start(out=tri_sb, in_=trim[:, :])
        wp_sb = []
        for i in range(2):
            t = consts.tile([128, D], BF, tag=f"wp{i}")
            nc.sync.dma_start(out=t, in_=wproj[i * 128:(i + 1) * 128, :])
            wp_sb.append(t)

        # persistent per-core tensors
        qrot = [bigs.tile([128, S], BF, tag=f"qrot{i}", name=f"qrot{i}")
                for i in range(2)]
        krot = [bigs.tile([128, S], BF, tag=f"krot{i}", name=f"krot{i}")
                for i in range(2)]
        v_sb = bigs.tile([128, 16 * VW], BF, tag="v")

        def proj_copy(ct, ps, cs, xvab, xvb2):
            if ct == 0:
                nc.vector.tensor_copy(qrot[0][:, cs], ps)
            elif ct == 1:
                nc.vector.tensor_copy(qrot[1][:, cs], ps)
            elif ct == 2:
                nc.vector.tensor_copy(krot[0][:, cs], ps)
            elif ct == 3:
                nc.vector.tensor_copy(krot[1][:, cs], ps)
            elif ct == 4:
                nc.vector.tensor_copy(xvab, ps)
            else:
                nc.vector.tensor_copy(xvb2[0:64, :], ps[0:64, :])
                nc.vector.memset(xvb2[64:65, :], 1.0)

        def phase_a_units(c):
            """Generator: projection / rope / v-decode for chunk c, one
            PE-sized unit of work per yield."""
            cs = slice(c * CH, (c + 1) * CH)
            xts = xts_pending.pop(0)
            if c + 1 < NCH:
                xts_pending.append(load_x(c + 1))
            xvab = xvpool.tile([128, CH], BF, tag="xvab", name="xvab")
            xvb2 = xvpool.tile([128, CH], BF, tag="xvb2", name="xvb2")
            for ct in range(6):
                ps = mmpool.tile([128, CH], F32, tag="mm", name="mm")
                m = 64 if ct == 5 else 128
                for kt in range(8):
                    nc.tensor.matmul(
                        ps[0:m, :],
                        lhsT=wcomb_sb[kt][:, ct * 128:ct * 128 + m],
                        rhs=xts[:, kt * CH:(kt + 1) * CH],
                        start=(kt == 0), stop=(kt == 7))
                proj_copy(ct, ps, cs, xvab, xvb2)
                yield
            # v decode: per 128-seq block, 4 heads side by side
            for j in range(4):
                sb = 4 * c + j
                js = slice(j * BLK, (j + 1) * BLK)
                psv = ypool.tile([128, CH], F32, tag="yp", name="psv")
                nc.tensor.matmul(psv[:, 0:VW], lhsT=xvab[:, js], rhs=vdeca_sb,
                                 start=True, stop=False)
                nc.tensor.matmul(psv[:, 0:VW], lhsT=xvb2[0:65, js],
                                 rhs=vdecb_sb[0:65, :], start=False, stop=True)
                if j % 2 == 0:
                    nc.scalar.copy(v_sb[:, sb * VW:(sb + 1) * VW],
                                   psv[:, 0:VW])
                else:
                    nc.vector.tensor_copy(v_sb[:, sb * VW:(sb + 1) * VW],
                                          psv[:, 0:VW])
                yield
            # RoPE: T <- T*cos + (P T)*sin, in T-layout per 512 chunk
            for tt in (qrot[0], qrot[1], krot[0], krot[1]):
                pq = mmpool.tile([128, CH], F32, tag="mm", name="mm")
                nc.tensor.matmul(pq, lhsT=perm_sb, rhs=tt[:, cs],
                                 start=True, stop=True)
                t1 = tmps.tile([128, CH], BF, tag="t1", name="t1")
                nc.vector.tensor_mul(t1, pq, sin_sb[:, cs])
                t2 = tmps.tile([128, CH], BF, tag="t2", name="t2")
                nc.vector.tensor_mul(t2, tt[:, cs], cos_sb[:, cs])
                nc.vector.tensor_add(tt[:, cs], t1, t2)
                yield

        def scores_units(c, h, ets):
            """Generator: scores + exp (+ causal mask) for head h, chunk c.
            Appends et tiles to ets; one psum tile per yield."""
            hp, hl = h // 2, h % 2
            rows = slice(hl * 64, (hl + 1) * 64)
            kk, qq = krot[hp], qrot[hp]
            cs = slice(c * CH, (c + 1) * CH)
            for g in range(2 * c):   # full block pairs
                sp = sppool.tile([128, 2 * CH], F32, tag="sp", name="sp")
                for i in range(2):
                    blk = 2 * g + i
                    nc.tensor.matmul(
                        sp[:, i * CH:(i + 1) * CH],
                        lhsT=kk[rows, blk * BLK:(blk + 1) * BLK],
                        rhs=qq[rows, cs], start=True, stop=True)
                et = epool.tile([128, 2 * CH], BF, tag="et", name="et")
                nc.scalar.activation(et, sp, AF.Exp, scale=0.125)
                ets.append(et)
                yield
            # diagonal blocks, N-shrunk: m covers cols [128m:512] of the chunk
            spd0 = sppool.tile([128, 2 * CH], F32, tag="sp", name="sp")
            nc.tensor.matmul(
                spd0[:, 0:512],
                lhsT=kk[rows, (4 * c) * BLK:(4 * c + 1) * BLK],
                rhs=qq[rows, cs], start=True, stop=True)
            nc.tensor.matmul(
                spd0[:, 512:896],
                lhsT=kk[rows, (4 * c + 1) * BLK:(4 * c + 2) * BLK],
                rhs=qq[rows, c * CH + 128:(c + 1) * CH], start=True, stop=True)
            etd0 = epool.tile([128, 2 * CH], BF, tag="et", name="et")
            nc.scalar.activation(etd0[:, 0:896], spd0[:, 0:896],
                                 AF.Exp, scale=0.125)
            nc.gpsimd.tensor_mul(etd0[:, 0:128], etd0[:, 0:128], tri_sb)
            nc.gpsimd.tensor_mul(etd0[:, 512:640], etd0[:, 512:640], tri_sb)
            ets.append(etd0)
            yield
            spd1 = sppool.tile([128, 2 * CH], F32, tag="sp", name="sp")
            nc.tensor.matmul(
                spd1[:, 0:256],
                lhsT=kk[rows, (4 * c + 2) * BLK:(4 * c + 3) * BLK],
                rhs=qq[rows, c * CH + 256:(c + 1) * CH], start=True, stop=True)
            nc.tensor.matmul(
                spd1[:, 256:384],
                lhsT=kk[rows, (4 * c + 3) * BLK:(4 * c + 4) * BLK],
                rhs=qq[rows, c * CH + 384:(c + 1) * CH], start=True, stop=True)
            etd1 = epool.tile([128, 2 * CH], BF, tag="et", name="et")
            nc.scalar.activation(etd1[:, 0:384], spd1[:, 0:384],
                                 AF.Exp, scale=0.125)
            nc.gpsimd.tensor_mul(etd1[:, 0:128], etd1[:, 0:128], tri_sb)
            nc.gpsimd.tensor_mul(etd1[:, 256:384], etd1[:, 256:384], tri_sb)
            ets.append(etd1)
            yield

        def attnv_units(c, h, ets, yn):
            hp, hl = h // 2, h % 2
            base = hp * 192 + hl * 64
            yp = ypool.tile([128, CH], F32, tag="yp", name="yp")
            for blk in range(4 * c):
                et = ets[blk // 2]
                off = (blk % 2) * CH
                nc.tensor.matmul(
                    yp, lhsT=v_sb[:, blk * VW + base:blk * VW + base + 128],
                    rhs=et[:, off:off + CH],
                    start=(blk == 0), stop=False)
                if blk % 2 == 1:
                    yield
            etd0, etd1 = ets[-2], ets[-1]
            # diag m writes cols [128m:512]; one start (first write) and one
            # stop (last write) per psum bank, column completion is tracked
            # by the tile framework's write deps
            dspec = [
                (0, etd0, 0, 512),
                (1, etd0, 512, 384),
                (2, etd1, 0, 256),
                (3, etd1, 256, 128),
            ]
            for m, etd, off, w in dspec:
                blk = 4 * c + m
                vsl = v_sb[:, blk * VW + base:blk * VW + base + 128]
                nc.tensor.matmul(
                    yp[:, m * 128:CH],
                    lhsT=vsl, rhs=etd[:, off:off + w],
                    start=(c == 0 and m == 0), stop=(m == 3))
                if m % 2 == 1:
                    yield
            # normalize: y * 1/denom. The 64 ones-columns replicated the
            # denominator onto the other 64 rows; reciprocal shifts it back
            # onto y's partitions (single-input ops may cross partitions,
            # tensor_tensor inputs may not).
            if hl == 0:
                yrows, drows = slice(0, 64), slice(64, 128)
            else:
                yrows, drows = slice(64, 128), slice(0, 64)
            rc = tmps.tile([128, CH], F32, tag="rc", name="rc")
            nc.vector.reciprocal(rc[yrows, :], yp[drows, :])
            nc.vector.tensor_mul(yn[hp][yrows, :], yp[yrows, :], rc[yrows, :])
            yield

        def b_units(c, yn):
            """Attention for chunk c: pair head h's scores with head h-1's
            attn@v so PE alternates between them while exps drain. Yields
            the head index after that head's normalize, else None."""
            prev = None
            for h in range(4):
                ets = []
                sg = scores_units(c, h, ets)
                ag = attnv_units(c, prev[0], prev[1], yn) if prev else None
                for _ in sg:
                    yield None
                    if ag is not None and next(ag, "done") != "done":
                        yield None
                if ag is not None:
                    for _ in ag:
                        yield None
                    yield prev[0]
                prev = (h, ets)
            for _ in attnv_units(c, prev[0], prev[1], yn):
                yield None
            yield prev[0]

        def out_proj_units(c, yn, alt_copies=False):
            for j in range(4):
                sb = 4 * c + j
                js = slice(j * BLK, (j + 1) * BLK)
                ot = otpool.tile([128, D], BF, tag="ot", name="ot")
                for n in range(2):
                    pp = mmpool.tile([128, CH], F32, tag="mm", name="mm")
                    nc.tensor.matmul(pp, lhsT=yn[0][:, js],
                                     rhs=wp_sb[0][:, n * CH:(n + 1) * CH],
                                     start=True, stop=False)
                    nc.tensor.matmul(pp, lhsT=yn[1][:, js],
                                     rhs=wp_sb[1][:, n * CH:(n + 1) * CH],
                                     start=False, stop=True)
                    if alt_copies and n == 1:
                        nc.scalar.copy(ot[:, n * CH:(n + 1) * CH], pp)
                    else:
                        nc.vector.tensor_copy(ot[:, n * CH:(n + 1) * CH], pp)
                    yield
                nc.sync.dma_start(
                    out=yout[sb * BLK:(sb + 1) * BLK, :], in_=ot)

        def out_proj_stage1(c, yn, ots):
            """First half of the projection (heads 0,1) for the final chunk:
            runs as soon as yn[0] is complete, while heads 2,3 attend."""
            for j in range(4):
                ot = otpool.tile([128, D], BF, tag="ot", name="ot")
                ots.append(ot)
                js = slice(j * BLK, (j + 1) * BLK)
                for n in range(2):
                    pp = mmpool.tile([128, CH], F32, tag="mm", name="mm")
                    nc.tensor.matmul(pp, lhsT=yn[0][:, js],
                                     rhs=wp_sb[0][:, n * CH:(n + 1) * CH],
                                     start=True, stop=True)
                    nc.vector.tensor_copy(ot[:, n * CH:(n + 1) * CH], pp)
                    yield

        def out_proj_stage2(c, yn, ots):
            for j in range(4):
                sb = 4 * c + j
                js = slice(j * BLK, (j + 1) * BLK)
                ot = ots[j]
                for n in range(2):
                    pp = mmpool.tile([128, CH], F32, tag="mm", name="mm")
                    nc.tensor.matmul(pp, lhsT=yn[1][:, js],
                                     rhs=wp_sb[1][:, n * CH:(n + 1) * CH],
                                     start=True, stop=True)
                    nc.vector.tensor_add(ot[:, n * CH:(n + 1) * CH],
                                         ot[:, n * CH:(n + 1) * CH], pp)
                    yield
                nc.sync.dma_start(
                    out=yout[sb * BLK:(sb + 1) * BLK, :], in_=ot)

        def drain_interleaved(gb, gas, ratio):
            """Emit all of gb, interspersing units from the ga generators at
            the given rate; then drain the gas."""
            acc = 0.0
            for _ in gb:
                acc += ratio
                while acc >= 1.0 and gas:
                    if next(gas[0], "done") == "done":
                        gas.pop(0)
                    else:
                        acc -= 1.0
            for ga in gas:
                for _ in ga:
                    pass

        # chunk 0 phase A runs alone (nothing to overlap with yet).
        # Output projections for chunks 0..2 are all deferred into the last
        # chunk's attention window: that window is Activation-bound (no next
        # phase A left to interleave), so it needs the PE filler the most.
        for _ in phase_a_units(0):
            pass
        yns = [[ynpool.tile([128, CH], BF, tag=f"yn{c}_{i}", name="yn")
                for i in range(2)] for c in range(NCH)]
        gb3 = b_units(NCH - 1, yns[NCH - 1])
        for c in range(NCH - 1):
            ga = phase_a_units(c + 1)
            a_done = False
            acc = 0.0
            ratio = 19.0 / (16 * c + 29)
            for _ in b_units(c, yns[c]):
                acc += ratio
                while acc >= 1.0 and not a_done:
                    if next(ga, "done") == "done":
                        a_done = True
                    else:
                        acc -= 1.0
                # once the last phase A is in, start the last chunk's
                # attention early: its exp stream is the global straggler
                if c == NCH - 2 and a_done:
                    next(gb3, "done")
                    next(gb3, "done")
            while not a_done:
                if next(ga, "done") == "done":
                    a_done = True
        gops = [out_proj_units(cc, yns[cc]) for cc in range(NCH - 1)]
        acc = 0.0
        for _ in gb3:
            acc += 0.6
            while acc >= 1.0 and gops:
                if next(gops[0], "done") == "done":
                    gops.pop(0)
                else:
                    acc -= 1.0
        for ga in gops:
            for _ in ga:
                pass
        for _ in out_proj_units(NCH - 1, yns[-1]):
            pass

    _split_dma_waits(nc, mybir)
    return nc


def _split_dma_waits(nc, mybir):
    """This container's walrus rejects instructions whose 64B encoding lacks
    room for their sem waits ("Too many sync wait commands"): DMAs and NoOps
    hold 1 wait, matmuls 2. Hoist excess waits onto a chain of single-wait
    NoOps in the same engine stream directly before the instruction — the
    sequencer blocks on each, which is semantically identical."""
    cap = {}
    f = nc.m.functions[0]
    blocks = f.body if hasattr(f, "body") else f.blocks
    n = 0
    for blk in blocks:
        insts = list(blk.instructions)
        out = []
        changed = False
        for inst in insts:
            si = inst.sync_info
            tn = type(inst).__name__
            limit = cap.get(tn, 1)
            if si is not None and si.on_wait and len(si.on_wait) > limit:
                waits = list(si.on_wait)
                keep = waits[-limit:]
                for w in waits[:-limit]:
                    nop = mybir.InstNoOp(name=f"I-dmaw-{n}")
                    n += 1
                    nop.engine = inst.engine
                    nop.sync_info = mybir.SyncInfo(on_wait=[w], on_update=[])
                    nc.register_instruction(nop)
                    out.append(nop)
                inst.sync_info = mybir.SyncInfo(
                    on_wait=keep, on_update=list(si.on_update or []))
                changed = True
            out.append(inst)
        if changed:
            if hasattr(blk, "set_instructions"):
                blk.set_instructions(out)
            else:
                try:
                    blk.instructions = out
                except Exception:
                    blk.instructions[:] = out
    return nc


def _host_inputs(x, Wq, Wk, Wv, key_decoder, value_decoder, Wproj):
    bf16 = ml_dtypes.bfloat16
    x = np.asarray(x, np.float32)
    Wq = np.asarray(Wq, np.float32)
    Wk = np.asarray(Wk, np.float32)
    Wv = np.asarray(Wv, np.float32)
    key_decoder = np.asarray(key_decoder, np.float32)
    value_decoder = np.asarray(value_decoder, np.float32)
    Wproj = np.asarray(Wproj, np.float32)

    xts = [np.ascontiguousarray(x[b].T).astype(bf16) for b in range(B)]

    half = HD // 2
    freq = 1.0 / (ROPE_BASE ** (np.arange(half, dtype=np.float32) / half))
    th = np.outer(np.arange(S, dtype=np.float32), freq)  # [S, 32]
    cos, sin = np.cos(th), np.sin(th)
    rows = np.arange(128)
    fidx = (rows % 64) // 2
    cosT = cos[:, fidx].T.astype(bf16)                       # [128, S]
    sgn = np.where(rows % 2 == 0, -1.0, 1.0)[:, None]
    sinT = (sin[:, fidx].T * sgn).astype(bf16)

    p = np.arange(128)
    permw = np.zeros((128, 128), np.float32)
    permw[p, p ^ 1] = 1.0       # pair swap
    permw = permw.astype(bf16)

    trim = (p[:, None] <= p[None, :]).astype(np.float32).astype(bf16)

    Wq4 = Wq.reshape(D, H, HD)
    br, bi = key_decoder[..., 0], key_decoder[..., 1]  # [F, H, KR]

    in_maps = []
    for core in range(NCORE):
        b, hg = core // 4, core % 4
        hh = [4 * hg + i for i in range(4)]

        wq0 = Wq4[:, [hh[0], hh[1]], :].reshape(D, 128)
        wq1 = Wq4[:, [hh[2], hh[3]], :].reshape(D, 128)
        # key decoder folded into Wk: kT = A^T Wk^T x = (Wk A)^T x
        kf = []
        for hp in range(2):
            A = np.zeros((192, 128), np.float32)
            for hl, h in enumerate((hh[2 * hp], hh[2 * hp + 1])):
                for f in range(F):
                    for r in range(KR):
                        A[f * 6 + r * 2 + 0, hl * 64 + 2 * f] = br[f, h, r]
                        A[f * 6 + r * 2 + 1, hl * 64 + 2 * f] = -bi[f, h, r]
                        A[f * 6 + r * 2 + 0, hl * 64 + 2 * f + 1] = bi[f, h, r]
                        A[f * 6 + r * 2 + 1, hl * 64 + 2 * f + 1] = br[f, h, r]
            kf.append(Wk @ A)    # [D, 128]
        wcomb = np.concatenate(
            [wq0, wq1, kf[0], kf[1], Wv], axis=1).astype(bf16)  # [D, 704]

        vdeca = np.zeros((128, VW), np.float32)
        vdecb = np.zeros((128, VW), np.float32)
        for i, h in enumerate(hh):
            hp, hl = i // 2, i % 2
            c0 = hp * 192 + hl * 128   # 0, 128, 192, 320
            vdeca[:, c0:c0 + 64] = value_decoder[h][0:128, :]
            vdecb[0:64, c0:c0 + 64] = value_decoder[h][128:192, :]
        vdecb[64, 64:128] = 1.0      # ones indicator rows
        vdecb[64, 256:320] = 1.0

        wproj_my = np.concatenate(
            [Wproj[h * 64:(h + 1) * 64, :] for h in hh], axis=0).astype(bf16)

        in_maps.append({
            "xt": xts[b], "wcomb": wcomb, "permw": permw,
            "vdeca": vdeca.astype(bf16), "vdecb": vdecb.astype(bf16),
            "wproj": wproj_my, "cosT": cosT, "sinT": sinT, "trim": trim,
        })
    return in_maps


def kernel(x, Wq, Wk, Wv, key_decoder, value_decoder, Wproj):
    from concourse.bass_utils import run_bass_kernel_spmd

    if "nc" not in _COMPILED:
        _COMPILED["nc"] = _build_bass()
    nc = _COMPILED["nc"]

    in_maps = _host_inputs(x, Wq, Wk, Wv, key_decoder, value_decoder, Wproj)
    import time as _time
    t0 = _time.time()
    res = run_bass_kernel_spmd(nc, in_maps, list(range(NCORE)))
    _COMPILED["exec_wall_ns"] = (_time.time() - t0) * 1e9
    _COMPILED["last_result"] = res
    out = np.zeros((B, S, D), np.float32)
    for core in range(NCORE):
        out[core // 4] += res.results[core]["yout"].astype(np.float32)
    return out


# revision 66
# speedup vs baseline: 1.7932x; 1.0111x over previous
"""Trainium2 Bass kernel for nn_CausalSelfAttention_45200235823551.

Causal self-attention with low-rank key/value encoders:
  D=1024, H=16 heads, HD=64, F=32 freqs, KR=3, VR=192, B=2, S=2048.

Sharding: 8 cores = 2 batches x 4 head-groups. Core i owns batch i//4 and
heads 4*(i%4)..4*(i%4)+3. Each core computes its heads' q/k/v, attention,
and a partial output projection (its heads' rows of Wproj); the host sums
the 4 partials per batch (row-parallel linear unshard).

Per-core layout ("T-major": feature rows on partitions, sequence on free):
  - xT [1024, 2048] bf16, loaded one 512-wide chunk per DMA
  - wcomb [1024, 704] = [Wq(2x128) | Wk@A key-fold (2x128) | Wv (128+64)]
    The key decoder is folded into the projection on the host:
    kT = A^T (Wk^T x) = (Wk A)^T x, so no separate decode stage.
  - RoPE pair-swap via a 128x128 permutation matmul (PE), then
    qrot = q*cos + (Pq)*sin on DVE with sign folded into the sin table
  - scoresT[sk,sq] = krotT.T @ qrotT per head (K=64)
  - causal N-shrink: diagonal block m only computes columns >= 128m;
    triangular mask is a [128,128] multiply on the first 128 columns
  - attn@v with v stationary: per head the 128-wide lhsT is
    [v(64) | ones(64)] so the other 64 accumulator rows replicate the
    softmax denominator; normalize = partition-shifted reciprocal + mul
  - partial projection: yout[sq,:] = yn.T @ Wproj_rows, bf16 partials
  - emission is software-pipelined: chunk c's attention interleaves with
    chunk c+1's projections (and, once those are in, the first units of
    chunk c+1's attention); all output projections are deferred into the
    final chunk's window, which has no phase A left to hide the exps
"""

import sys

import numpy as np

sys.path.insert(0, "/opt/trn_rl_repo")

import ml_dtypes

D, H, HD = 1024, 16, 64
F, KR, VR = 32, 3, 192
B, S = 2, 2048
NCORE = 8
CH = 512          # sq chunk width
NCH = S // CH     # 4
BLK = 128         # sk block
VW = 384          # v_sb per-block: [v_h0|ones|v_h1 | v_h2|ones|v_h3]
ROPE_BASE = 10000.0

_COMPILED = {}


def _build_bass():
    import concourse.bass as bass
    import concourse.tile as tile
    from concourse import mybir
    from contextlib import ExitStack

    BF = mybir.dt.bfloat16
    F32 = mybir.dt.float32
    AF = mybir.ActivationFunctionType

    nc = bass.Bass()
    xt = nc.dram_tensor("xt", [D, S], BF, kind="ExternalInput")
    wcomb = nc.dram_tensor("wcomb", [D, 704], BF, kind="ExternalInput")
    permw = nc.dram_tensor("permw", [128, 128], BF, kind="ExternalInput")
    vdeca = nc.dram_tensor("vdeca", [128, VW], BF, kind="ExternalInput")
    vdecb = nc.dram_tensor("vdecb", [128, VW], BF, kind="ExternalInput")
    wproj = nc.dram_tensor("wproj", [256, D], BF, kind="ExternalInput")
    cosT = nc.dram_tensor("cosT", [128, S], BF, kind="ExternalInput")
    sinT = nc.dram_tensor("sinT", [128, S], BF, kind="ExternalInput")
    trim = nc.dram_tensor("trim", [128, 128], BF, kind="ExternalInput")
    yout = nc.dram_tensor("yout", [S, D], BF, kind="ExternalOutput")

    with ExitStack() as ctx:
        tc = ctx.enter_context(tile.TileContext(nc))
        consts = ctx.enter_context(tc.tile_pool(name="consts", bufs=1))
        bigs = ctx.enter_context(tc.tile_pool(name="bigs", bufs=1))
        xpool = ctx.enter_context(tc.tile_pool(name="xpool", bufs=2))
        xvpool = ctx.enter_context(tc.tile_pool(name="xvpool", bufs=2))
        tmps = ctx.enter_context(tc.tile_pool(name="tmps", bufs=4))
        ynpool = ctx.enter_context(tc.tile_pool(name="ynpool", bufs=1))
        epool = ctx.enter_context(tc.tile_pool(name="epool", bufs=26))
        otpool = ctx.enter_context(tc.tile_pool(name="otpool", bufs=7))
        mmpool = ctx.enter_context(tc.tile_pool(name="mmpool", bufs=2, space="PSUM"))
        sppool = ctx.enter_context(tc.tile_pool(name="sppool", bufs=2, space="PSUM"))
        ypool = ctx.enter_context(tc.tile_pool(name="ypool", bufs=2, space="PSUM"))

        # ---- input prefetch + constants, ordered for fastest PE start ----
        def load_x(c):
            cs = slice(c * CH, (c + 1) * CH)
            t = xpool.tile([128, 8 * CH], BF, tag="xts", name="xts")
            nc.sync.dma_start(
                out=t[:, :].rearrange("p (k j) -> p k j", k=8, j=CH),
                in_=xt[:, cs].rearrange("(k p) j -> p k j", k=8, p=128))
            return t

        # chunk 0: x first (the longest transfer), weight tiles stream
        # in behind it and the first ct group consumes them as they land
        xts_pending = [load_x(0)]
        wcomb_sb = []
        for kt in range(8):
            t = consts.tile([128, 704], BF, tag=f"wcomb{kt}")
            nc.sync.dma_start(out=t, in_=wcomb[kt * 128:(kt + 1) * 128, :])
            wcomb_sb.append(t)
        perm_sb = consts.tile([128, 128], BF, tag="perm")
        nc.sync.dma_start(out=perm_sb, in_=permw[:, :])
        cos_sb = consts.tile([128, S], BF, tag="cos")
        nc.sync.dma_start(out=cos_sb, in_=cosT[:, :])
        sin_sb = consts.tile([128, S], BF, tag="sin")
        nc.sync.dma_start(out=sin_sb, in_=sinT[:, :])
        vdeca_sb = consts.tile([128, VW], BF, tag="vdeca")
        nc.sync.dma_start(out=vdeca_sb, in_=vdeca[:, :])
        vdecb_sb = consts.tile([128, VW], BF, tag="vdecb")
        nc.sync.dma_start(out=vdecb_sb, in_=vdecb[:, :])
        tri_sb = consts.tile([128, 128], BF, tag="tri")
        nc.sync.dma_start(out=tri_sb, in_=trim[:, :])
        wp_sb = []
        for i in range(2):
            t = consts.tile([128, D], BF, tag=f"wp{i}")
            nc.sync.dma_start(out=t, in_=wproj[i * 128:(i + 1) * 128, :])
            wp_sb.append(t)

        # persistent per-core tensors
        qrot = [bigs.tile([128, S], BF, tag=f"qrot{i}", name=f"qrot{i}")
                for i in range(2)]
        krot = [bigs.tile([128, S], BF, tag=f"krot{i}", name=f"krot{i}")
                for i in range(2)]
        v_sb = bigs.tile([128, 16 * VW], BF, tag="v")

        def proj_copy(ct, ps, cs, xvab, xvb2):
            if ct == 0:
                nc.vector.tensor_copy(qrot[0][:, cs], ps)
            elif ct == 1:
                nc.vector.tensor_copy(qrot[1][:, cs], ps)
            elif ct == 2:
                nc.vector.tensor_copy(krot[0][:, cs], ps)
            elif ct == 3:
                nc.vector.tensor_copy(krot[1][:, cs], ps)
            elif ct == 4:
                nc.vector.tensor_copy(xvab, ps)
            else:
                nc.vector.tensor_copy(xvb2[0:64, :], ps[0:64, :])
                nc.vector.memset(xvb2[64:65, :], 1.0)

        def phase_a_units(c):
            """Generator: projection / rope / v-decode for chunk c, one
            PE-sized unit of work per yield."""
            cs = slice(c * CH, (c + 1) * CH)
            xts = xts_pending.pop(0)
            if c + 1 < NCH:
                xts_pending.append(load_x(c + 1))
            xvab = xvpool.tile([128, CH], BF, tag="xvab", name="xvab")
            xvb2 = xvpool.tile([128, CH], BF, tag="xvb2", name="xvb2")
            for ct in range(6):
                ps = mmpool.tile([128, CH], F32, tag="mm", name="mm")
                m = 64 if ct == 5 else 128
                for kt in range(8):
                    nc.tensor.matmul(
                        ps[0:m, :],
                        lhsT=wcomb_sb[kt][:, ct * 128:ct * 128 + m],
                        rhs=xts[:, kt * CH:(kt + 1) * CH],
                        start=(kt == 0), stop=(kt == 7))
                proj_copy(ct, ps, cs, xvab, xvb2)
                yield
            # v decode: per 128-seq block, 4 heads side by side
            for j in range(4):
                sb = 4 * c + j
                js = slice(j * BLK, (j + 1) * BLK)
                psv = ypool.tile([128, CH], F32, tag="yp", name="psv")
                nc.tensor.matmul(psv[:, 0:VW], lhsT=xvab[:, js], rhs=vdeca_sb,
                                 start=True, stop=False)
                nc.tensor.matmul(psv[:, 0:VW], lhsT=xvb2[0:65, js],
                                 rhs=vdecb_sb[0:65, :], start=False, stop=True)
                if j % 2 == 0:
                    nc.scalar.copy(v_sb[:, sb * VW:(sb + 1) * VW],
                                   psv[:, 0:VW])
                else:
                    nc.vector.tensor_copy(v_sb[:, sb * VW:(sb + 1) * VW],
                                          psv[:, 0:VW])
                yield
            # RoPE: T <- T*cos + (P T)*sin, in T-layout per 512 chunk
            for tt in (qrot[0], qrot[1], krot[0], krot[1]):
                pq = mmpool.tile([128, CH], F32, tag="mm", name="mm")
                nc.tensor.matmul(pq, lhsT=perm_sb, rhs=tt[:, cs],
                                 start=True, stop=True)
                t1 = tmps.tile([128, CH], BF, tag="t1", name="t1")
                nc.vector.tensor_mul(t1, pq, sin_sb[:, cs])
                t2 = tmps.tile([128, CH], BF, tag="t2", name="t2")
                nc.vector.tensor_mul(t2, tt[:, cs], cos_sb[:, cs])
                nc.vector.tensor_add(tt[:, cs], t1, t2)
                yield

        def scores_units(c, h, ets):
            """Generator: scores + exp (+ causal mask) for head h, chunk c.
            Appends et tiles to ets; one psum tile per yield."""
            hp, hl = h // 2, h % 2
            rows = slice(hl * 64, (hl + 1) * 64)
            kk, qq = krot[hp], qrot[hp]
            cs = slice(c * CH, (c + 1) * CH)
            for g in range(2 * c):   # full block pairs
                sp = sppool.tile([128, 2 * CH], F32, tag="sp", name="sp")
                for i in range(2):
                    blk = 2 * g + i
                    nc.tensor.matmul(
                        sp[:, i * CH:(i + 1) * CH],
                        lhsT=kk[rows, blk * BLK:(blk + 1) * BLK],
                        rhs=qq[rows, cs], start=True, stop=True)
                et = epool.tile([128, 2 * CH], BF, tag="et", name="et")
                nc.scalar.activation(et, sp, AF.Exp, scale=0.125)
                ets.append(et)
                yield
            # diagonal blocks, N-shrunk: m covers cols [128m:512] of the chunk
            spd0 = sppool.tile([128, 2 * CH], F32, tag="sp", name="sp")
            nc.tensor.matmul(
                spd0[:, 0:512],
                lhsT=kk[rows, (4 * c) * BLK:(4 * c + 1) * BLK],
                rhs=qq[rows, cs], start=True, stop=True)
            nc.tensor.matmul(
                spd0[:, 512:896],
                lhsT=kk[rows, (4 * c + 1) * BLK:(4 * c + 2) * BLK],
                rhs=qq[rows, c * CH + 128:(c + 1) * CH], start=True, stop=True)
            etd0 = epool.tile([128, 2 * CH], BF, tag="et", name="et")
            nc.scalar.activation(etd0[:, 0:896], spd0[:, 0:896],
                                 AF.Exp, scale=0.125)
            nc.gpsimd.tensor_mul(etd0[:, 0:128], etd0[:, 0:128], tri_sb)
            nc.gpsimd.tensor_mul(etd0[:, 512:640], etd0[:, 512:640], tri_sb)
            ets.append(etd0)
            yield
            spd1 = sppool.tile([128, 2 * CH], F32, tag="sp", name="sp")
            nc.tensor.matmul(
                spd1[:, 0:256],
                lhsT=kk[rows, (4 * c + 2) * BLK:(4 * c + 3) * BLK],
                rhs=qq[rows, c * CH + 256:(c + 1) * CH], start=True, stop=True)
            nc.tensor.matmul(
                spd1[:, 256:384],
                lhsT=kk[rows, (4 * c + 3) * BLK:(4 * c + 4) * BLK],
                rhs=qq[rows, c * CH + 384:(c + 1) * CH], start=True, stop=True)
            etd1 = epool.tile([128, 2 * CH], BF, tag="et", name="et")
            nc.scalar.activation(etd1[:, 0:384], spd1[:, 0:384],
                                 AF.Exp, scale=0.125)
            nc.gpsimd.tensor_mul(etd1[:, 0:128], etd1[:, 0:128], tri_sb)
            nc.gpsimd.tensor_mul(etd1[:, 256:384], etd1[:, 256:384], tri_sb)
            ets.append(etd1)
            yield

        def attnv_units(c, h, ets, yn):
            hp, hl = h // 2, h % 2
            base = hp * 192 + hl * 64
            yp = ypool.tile([128, CH], F32, tag="yp", name="yp")
            for blk in range(4 * c):
                et = ets[blk // 2]
                off = (blk % 2) * CH
                nc.tensor.matmul(
                    yp, lhsT=v_sb[:, blk * VW + base:blk * VW + base + 128],
                    rhs=et[:, off:off + CH],
                    start=(blk == 0), stop=False)
                if blk % 2 == 1:
                    yield
            etd0, etd1 = ets[-2], ets[-1]
            # diag m writes cols [128m:512]; one start (first write) and one
            # stop (last write) per psum bank, column completion is tracked
            # by the tile framework's write deps
            dspec = [
                (0, etd0, 0, 512),
                (1, etd0, 512, 384),
                (2, etd1, 0, 256),
                (3, etd1, 256, 128),
            ]
            for m, etd, off, w in dspec:
                blk = 4 * c + m
                vsl = v_sb[:, blk * VW + base:blk * VW + base + 128]
                nc.tensor.matmul(
                    yp[:, m * 128:CH],
                    lhsT=vsl, rhs=etd[:, off:off + w],
                    start=(c == 0 and m == 0), stop=(m == 3))
                if m % 2 == 1:
                    yield
            # normalize: y * 1/denom. The 64 ones-columns replicated the
            # denominator onto the other 64 rows; reciprocal shifts it back
            # onto y's partitions (single-input ops may cross partitions,
            # tensor_tensor inputs may not).
            if hl == 0:
                yrows, drows = slice(0, 64), slice(64, 128)
            else:
                yrows, drows = slice(64, 128), slice(0, 64)
            rc = tmps.tile([128, CH], F32, tag="rc", name="rc")
            nc.vector.reciprocal(rc[yrows, :], yp[drows, :])
            nc.vector.tensor_mul(yn[hp][yrows, :], yp[yrows, :], rc[yrows, :])
            yield

        def b_units(c, yn):
            """Attention for chunk c: pair head h's scores with head h-1's
            attn@v so PE alternates between them while exps drain. Yields
            the head index after that head's normalize, else None."""
            prev = None
            for h in range(4):
                ets = []
                sg = scores_units(c, h, ets)
                ag = attnv_units(c, prev[0], prev[1], yn) if prev else None
                for _ in sg:
                    yield None
                    if ag is not None and next(ag, "done") != "done":
                        yield None
                if ag is not None:
                    for _ in ag:
                        yield None
                    yield prev[0]
                prev = (h, ets)
            for _ in attnv_units(c, prev[0], prev[1], yn):
                yield None
            yield prev[0]

        def out_proj_units(c, yn, split_dma=False):
            for j in range(4):
                sb = 4 * c + j
                js = slice(j * BLK, (j + 1) * BLK)
                ot = otpool.tile([128, D], BF, tag="ot", name="ot")
                for n in range(2):
                    pp = mmpool.tile([128, CH], F32, tag="mm", name="mm")
                    nc.tensor.matmul(pp, lhsT=yn[0][:, js],
                                     rhs=wp_sb[0][:, n * CH:(n + 1) * CH],
                                     start=True, stop=False)
                    nc.tensor.matmul(pp, lhsT=yn[1][:, js],
                                     rhs=wp_sb[1][:, n * CH:(n + 1) * CH],
                                     start=False, stop=True)
                    nc.vector.tensor_copy(ot[:, n * CH:(n + 1) * CH], pp)
                    if split_dma:
                        nc.sync.dma_start(
                            out=yout[sb * BLK:(sb + 1) * BLK,
                                     n * CH:(n + 1) * CH],
                            in_=ot[:, n * CH:(n + 1) * CH])
                    yield
                if not split_dma:
                    nc.sync.dma_start(
                        out=yout[sb * BLK:(sb + 1) * BLK, :], in_=ot)

        # chunk 0 phase A runs alone (nothing to overlap with yet).
        # Output projections for chunks 0..2 are all deferred into the last
        # chunk's attention window: that window is Activation-bound (no next
        # phase A left to interleave), so it needs the PE filler the most.
        for _ in phase_a_units(0):
            pass
        yns = [[ynpool.tile([128, CH], BF, tag=f"yn{c}_{i}", name="yn")
                for i in range(2)] for c in range(NCH)]
        gbs = [b_units(c, yns[c]) for c in range(NCH)]
        for c in range(NCH - 1):
            ga = phase_a_units(c + 1)
            a_done = False
            acc = 0.0
            ratio = 19.0 / (16 * c + 29)
            for _ in gbs[c]:
                acc += ratio
                while acc >= 1.0 and not a_done:
                    if next(ga, "done") == "done":
                        a_done = True
                    else:
                        acc -= 1.0
                # once the next phase A is fully in, pull the next chunk's
                # attention forward: its exp stream lags the PE otherwise
                if a_done:
                    next(gbs[c + 1], "done")
                    if c == NCH - 2:
                        next(gbs[c + 1], "done")
            while not a_done:
                if next(ga, "done") == "done":
                    a_done = True
        gops = [out_proj_units(cc, yns[cc]) for cc in range(NCH - 1)]
        acc = 0.0
        for _ in gbs[NCH - 1]:
            acc += 0.6
            while acc >= 1.0 and gops:
                if next(gops[0], "done") == "done":
                    gops.pop(0)
                else:
                    acc -= 1.0
        for ga in gops:
            for _ in ga:
                pass
        for _ in out_proj_units(NCH - 1, yns[-1], split_dma=True):
            pass

    _split_dma_waits(nc, mybir)
    return nc


def _split_dma_waits(nc, mybir):
    """This container's walrus rejects instructions whose 64B encoding lacks
    room for their sem waits ("Too many sync wait commands"): DMAs and NoOps
    hold 1 wait, matmuls 2. Hoist excess waits onto a chain of single-wait
    NoOps in the same engine stream directly before the instruction — the
    sequencer blocks on each, which is semantically identical."""
    cap = {}
    f = nc.m.functions[0]
    blocks = f.body if hasattr(f, "body") else f.blocks
    n = 0
    for blk in blocks:
        insts = list(blk.instructions)
        out = []
        changed = False
        for inst in insts:
            si = inst.sync_info
            tn = type(inst).__name__
            limit = cap.get(tn, 1)
            if si is not None and si.on_wait and len(si.on_wait) > limit:
                waits = list(si.on_wait)
                keep = waits[-limit:]
                for w in waits[:-limit]:
                    nop = mybir.InstNoOp(name=f"I-dmaw-{n}")
                    n += 1
                    nop.engine = inst.engine
                    nop.sync_info = mybir.SyncInfo(on_wait=[w], on_update=[])
                    nc.register_instruction(nop)
                    out.append(nop)
                inst.sync_info = mybir.SyncInfo(
                    on_wait=keep, on_update=list(si.on_update or []))
                changed = True
            out.append(inst)
        if changed:
            if hasattr(blk, "set_instructions"):
                blk.set_instructions(out)
            else:
                try:
                    blk.instructions = out
                except Exception:
                    blk.instructions[:] = out
    return nc


def _host_inputs(x, Wq, Wk, Wv, key_decoder, value_decoder, Wproj):
    bf16 = ml_dtypes.bfloat16
    x = np.asarray(x, np.float32)
    Wq = np.asarray(Wq, np.float32)
    Wk = np.asarray(Wk, np.float32)
    Wv = np.asarray(Wv, np.float32)
    key_decoder = np.asarray(key_decoder, np.float32)
    value_decoder = np.asarray(value_decoder, np.float32)
    Wproj = np.asarray(Wproj, np.float32)

    xts = [np.ascontiguousarray(x[b].T).astype(bf16) for b in range(B)]

    half = HD // 2
    freq = 1.0 / (ROPE_BASE ** (np.arange(half, dtype=np.float32) / half))
    th = np.outer(np.arange(S, dtype=np.float32), freq)  # [S, 32]
    cos, sin = np.cos(th), np.sin(th)
    rows = np.arange(128)
    fidx = (rows % 64) // 2
    cosT = cos[:, fidx].T.astype(bf16)                       # [128, S]
    sgn = np.where(rows % 2 == 0, -1.0, 1.0)[:, None]
    sinT = (sin[:, fidx].T * sgn).astype(bf16)

    p = np.arange(128)
    permw = np.zeros((128, 128), np.float32)
    permw[p, p ^ 1] = 1.0       # pair swap
    permw = permw.astype(bf16)

    trim = (p[:, None] <= p[None, :]).astype(np.float32).astype(bf16)

    Wq4 = Wq.reshape(D, H, HD)
    br, bi = key_decoder[..., 0], key_decoder[..., 1]  # [F, H, KR]

    in_maps = []
    for core in range(NCORE):
        b, hg = core // 4, core % 4
        hh = [4 * hg + i for i in range(4)]

        wq0 = Wq4[:, [hh[0], hh[1]], :].reshape(D, 128)
        wq1 = Wq4[:, [hh[2], hh[3]], :].reshape(D, 128)
        # key decoder folded into Wk: kT = A^T Wk^T x = (Wk A)^T x
        kf = []
        for hp in range(2):
            A = np.zeros((192, 128), np.float32)
            for hl, h in enumerate((hh[2 * hp], hh[2 * hp + 1])):
                for f in range(F):
                    for r in range(KR):
                        A[f * 6 + r * 2 + 0, hl * 64 + 2 * f] = br[f, h, r]
                        A[f * 6 + r * 2 + 1, hl * 64 + 2 * f] = -bi[f, h, r]
                        A[f * 6 + r * 2 + 0, hl * 64 + 2 * f + 1] = bi[f, h, r]
                        A[f * 6 + r * 2 + 1, hl * 64 + 2 * f + 1] = br[f, h, r]
            kf.append(Wk @ A)    # [D, 128]
        wcomb = np.concatenate(
            [wq0, wq1, kf[0], kf[1], Wv], axis=1).astype(bf16)  # [D, 704]

        vdeca = np.zeros((128, VW), np.float32)
        vdecb = np.zeros((128, VW), np.float32)
        for i, h in enumerate(hh):
            hp, hl = i // 2, i % 2
            c0 = hp * 192 + hl * 128   # 0, 128, 192, 320
            vdeca[:, c0:c0 + 64] = value_decoder[h][0:128, :]
            vdecb[0:64, c0:c0 + 64] = value_decoder[h][128:192, :]
        vdecb[64, 64:128] = 1.0      # ones indicator rows
        vdecb[64, 256:320] = 1.0

        wproj_my = np.concatenate(
            [Wproj[h * 64:(h + 1) * 64, :] for h in hh], axis=0).astype(bf16)

        in_maps.append({
            "xt": xts[b], "wcomb": wcomb, "permw": permw,
            "vdeca": vdeca.astype(bf16), "vdecb": vdecb.astype(bf16),
            "wproj": wproj_my, "cosT": cosT, "sinT": sinT, "trim": trim,
        })
    return in_maps


def kernel(x, Wq, Wk, Wv, key_decoder, value_decoder, Wproj):
    from concourse.bass_utils import run_bass_kernel_spmd

    if "nc" not in _COMPILED:
        _COMPILED["nc"] = _build_bass()
    nc = _COMPILED["nc"]

    in_maps = _host_inputs(x, Wq, Wk, Wv, key_decoder, value_decoder, Wproj)
    import time as _time
    t0 = _time.time()
    res = run_bass_kernel_spmd(nc, in_maps, list(range(NCORE)))
    _COMPILED["exec_wall_ns"] = (_time.time() - t0) * 1e9
    _COMPILED["last_result"] = res
    out = np.zeros((B, S, D), np.float32)
    for core in range(NCORE):
        out[core // 4] += res.results[core]["yout"].astype(np.float32)
    return out
